# revision 12
# baseline (speedup 1.0000x reference)
"""Trainium2 Bass kernel for a binarized BasicBlock (2x bconv3x3 + BN +
residual hardtanh + channel shuffle), data-parallel over batch on 8 cores.

Self-contained: hardcodes shapes from the problem spec.
  x: (32, 256, 56, 56) f32 -> out: (32, 256, 56, 56) f32

Layout strategy:
- activations/residuals kept compact [128, 56, 56] in SBUF (efficient DMA)
- only the binarized conv operands live in a zero/half-padded 58-wide
  layout [128, 59, 58]; the binarize op itself (is_ge -> {0,1} "u-domain",
  pads 0.5, correction folded into BN bias on host) does the spread.
- conv = 9 accumulating matmuls (one per tap) into PSUM [128, 464] per
  8-row tile; junk at cols 56/57 of each row is never consumed.
- both channel_shuffles are free: host-permuted w2 channels + stride-4
  channel DMA writes.
"""

import numpy as np
import ml_dtypes

import concourse.bass as bass
import concourse.tile as tile
from concourse import bacc, mybir
from concourse import bass_utils

EPS = 1e-5
P = 128
H = W = 56
WP = 58          # padded row width
RP = 59          # padded rows allocated (58 used + 1 spare for tail matmul reads)
IMGS_PER_CORE = 4
NCORES = 8
NT = 7           # row tiles per image (8 output rows each)
TF = 8 * WP      # matmul free size per tile = 464
CF = 8 * W       # compact free size per tile = 448

F32 = mybir.dt.float32
BF16 = mybir.dt.bfloat16
ALU = mybir.AluOpType
ACTF = mybir.ActivationFunctionType

_CACHE = {}


def _flat(ap3):
    return ap3.rearrange("p r c -> p (r c)")


def _build():
    nc = bacc.Bacc("TRN2", target_bir_lowering=False, debug=False)

    x_h = nc.dram_tensor("xs", [IMGS_PER_CORE, 2 * P, H, W], F32, kind="ExternalInput")
    w1_h = nc.dram_tensor("w1m", [P, 9 * P], BF16, kind="ExternalInput")
    w2_h = nc.dram_tensor("w2m", [P, 9 * P], BF16, kind="ExternalInput")
    cst_h = nc.dram_tensor("cst", [P, 16], F32, kind="ExternalInput")
    out_h = nc.dram_tensor("out", [IMGS_PER_CORE, 2 * P, H, W], F32, kind="ExternalOutput")

    x_ap = x_h.ap()

    def out_ch4(n, base_ch, nch=64, half=None):
        # DRAM AP: channels base_ch, base_ch+4, ... of image n, full spatial
        # (or the first/second 32-row half when half is 0/1).
        off, sz = 0, H * W
        if half is not None:
            off = half * 32 * W
            sz = 32 * W if half == 0 else (H - 32) * W
        return bass.AP(
            tensor=out_h,
            offset=(n * 2 * P + base_ch) * H * W + off,
            ap=[[4 * H * W, nch], [1, sz]],
        )

    with tile.TileContext(nc) as tc:
        # persistent ping-pong buffers (compact except B1/B2)
        XA = [nc.alloc_sbuf_tensor(f"XA{i}", [P, H, W], F32).ap() for i in range(2)]
        A2 = [nc.alloc_sbuf_tensor(f"A2{i}", [P, H, W], F32).ap() for i in range(2)]
        B1 = [nc.alloc_sbuf_tensor(f"B1{i}", [P, RP, WP], BF16).ap() for i in range(2)]
        B2 = [nc.alloc_sbuf_tensor(f"B2{i}", [P, RP, WP], BF16).ap() for i in range(2)]
        XIH = [nc.alloc_sbuf_tensor(f"XIH{i}", [P, H, W], F32).ap() for i in range(2)]
        FO1 = [nc.alloc_sbuf_tensor(f"FO1{i}", [P, H, W], F32).ap() for i in range(2)]
        OT2 = [nc.alloc_sbuf_tensor(f"OT2{i}", [P, H, W], F32).ap() for i in range(2)]
        WS1 = nc.alloc_sbuf_tensor("WS1", [P, 9 * P], BF16).ap()
        WS2 = nc.alloc_sbuf_tensor("WS2", [P, 9 * P], BF16).ap()
        CST = nc.alloc_sbuf_tensor("CST", [P, 16], F32).ap()

        s1 = CST[:, 0:1]
        b1 = CST[:, 1:2]
        s2 = CST[:, 2:3]
        b2 = CST[:, 3:4]
        beta_hi = CST[64:128, 4:5]
        mv0_lo = CST[64:128, 7:8]
        cxh = CST[64:128, 8:9]

        nc.sync.dma_start(out=WS1, in_=w1_h.ap())
        nc.sync.dma_start(out=WS2, in_=w2_h.ap())
        nc.sync.dma_start(out=CST, in_=cst_h.ap())

        # u-domain pads: 0.5 stands for binarized zero-padding. Interiors
        # are rewritten per image; pads never touched again, so only the
        # pad region is initialized (top row, bottom rows, side columns).
        for _b in (*B1, *B2):
            _f = _flat(_b)
            nc.gpsimd.memset(_f[:, 0:WP], 0.5)
            nc.gpsimd.memset(_f[:, 57 * WP:RP * WP], 0.5)
            nc.gpsimd.memset(_b[:, 1:57, 0:1], 0.5)
            nc.gpsimd.memset(_b[:, 1:57, 57:58], 0.5)

        with (
            tc.tile_pool(name="psum1", bufs=2, space="PSUM") as psum1_pool,
            tc.tile_pool(name="psum2", bufs=2, space="PSUM") as psum2_pool,
            tc.tile_pool(name="stage", bufs=4) as stage_pool,
        ):
            def xa_load(n):
                """Load x_act for image n (emit as soon as the XA slot's
                last reader, conv1(n-2), has been emitted)."""
                s = n % 2
                xa = XA[s]
                if n == 0:
                    # head: chunk the first load so conv1(0) starts sooner
                    nc.sync.dma_start(out=xa[:, 0:32], in_=x_ap[n, 0:P, 0:32])
                    nc.sync.dma_start(out=xa[:, 32:56], in_=x_ap[n, 0:P, 32:56])
                else:
                    nc.sync.dma_start(out=xa, in_=x_ap[n, 0:P])

            def u1(n):
                """Binarize x_act into padded B1 (2 chunks)."""
                s = n % 2
                xa = XA[s]
                nc.vector.tensor_scalar(
                    out=B1[s][:, 1:33, 1:57], in0=xa[:, 0:32],
                    scalar1=0.0, scalar2=None, op0=ALU.is_ge)
                nc.vector.tensor_scalar(
                    out=B1[s][:, 33:57, 1:57], in0=xa[:, 32:56],
                    scalar1=0.0, scalar2=None, op0=ALU.is_ge)

            def prelude_idle_loads(n):
                """Idle-half loads for image n (must trail conv2(n-2),
                which reads the same A2 slot)."""
                s = n % 2
                nc.sync.dma_start(out=A2[s][64:128], in_=x_ap[n, P:P + 64])
                nc.sync.dma_start(out=XIH[s][64:128], in_=x_ap[n, P + 64:2 * P])

            def idle_bias_chunk(n, i):
                """One quarter of the idle-half bias work, interleaved into
                conv1(n)'s tile loop to keep the ACT queue free of blobs."""
                s = n % 2
                a2, xih = A2[s], XIH[s]
                if i == 0:
                    nc.scalar.activation(
                        a2[64:128, 0:28], a2[64:128, 0:28],
                        ACTF.Identity, bias=mv0_lo)
                elif i == 1:
                    nc.scalar.activation(
                        a2[64:128, 28:56], a2[64:128, 28:56],
                        ACTF.Identity, bias=mv0_lo)
                elif i == 2:
                    nc.scalar.activation(
                        xih[64:128, 0:28], xih[64:128, 0:28],
                        ACTF.Identity, bias=cxh)
                else:
                    nc.scalar.activation(
                        xih[64:128, 28:56], xih[64:128, 28:56],
                        ACTF.Identity, bias=cxh)
                    nc.scalar.dma_start(out=out_ch4(n, 3), in_=_flat(xih)[64:128])

            def conv1(n):
                s = n % 2
                xa, a2, fo1 = XA[s], A2[s], FO1[s]
                b1f = _flat(B1[s])
                for tp in range(4):  # tile pairs (0,1),(2,3),(4,5),(6,)
                    npair = 2 if tp < 3 else 1
                    nr = 8 * npair
                    r0 = 16 * tp
                    idle_bias_chunk(n, tp)
                    if tp == 2 and n + 1 < IMGS_PER_CORE:
                        u1(n + 1)
                    ps = psum1_pool.tile([P, 2, 512], F32)
                    for j in range(npair):
                        t = 2 * tp + j
                        for k in range(9):
                            ky, kx = divmod(k, 3)
                            off = (8 * t + ky) * WP + kx
                            nc.tensor.matmul(
                                ps[:, j, 0:TF], lhsT=WS1[:, P * k:P * (k + 1)],
                                rhs=b1f[:, off:off + TF],
                                start=(k == 0), stop=(k == 8))
                    t1 = stage_pool.tile([P, 2, TF], F32, tag="t1")
                    # rows of the pair are uniformly 58-strided (464 = 8*58)
                    t13 = _flat(t1).rearrange("p (r c) -> p r c", c=WP)[:, 0:nr, 0:W]
                    nc.scalar.activation(
                        t1[:, 0:npair], ps[:, 0:npair, 0:TF],
                        ACTF.Identity, bias=b1, scale=s1)
                    nc.vector.tensor_tensor(
                        out=t13, in0=t13, in1=xa[:, r0:r0 + nr, :], op=ALU.add)
                    # lo channels -> A2 (conv2 input/residual)
                    nc.gpsimd.tensor_scalar(
                        out=a2[0:64, r0:r0 + nr, :], in0=t13[0:64],
                        scalar1=1.0, scalar2=-1.0, op0=ALU.min, op1=ALU.max)
                    # hi channels: clip in place, then +move1_even -> FO1
                    nc.gpsimd.tensor_scalar(
                        out=t13[64:128], in0=t13[64:128],
                        scalar1=1.0, scalar2=-1.0, op0=ALU.min, op1=ALU.max)
                    nc.vector.tensor_scalar(
                        out=fo1[64:128, r0:r0 + nr, :], in0=t13[64:128],
                        scalar1=beta_hi, scalar2=None, op0=ALU.add)
                nc.scalar.dma_start(out=out_ch4(n, 1), in_=_flat(fo1)[64:128])
                # u2 = (a2 >= 0) in {0,1}, spread into padded B2 (2 chunks)
                nc.vector.tensor_scalar(
                    out=B2[s][:, 1:33, 1:57], in0=a2[:, 0:32],
                    scalar1=0.0, scalar2=None, op0=ALU.is_ge)
                nc.vector.tensor_scalar(
                    out=B2[s][:, 33:57, 1:57], in0=a2[:, 32:56],
                    scalar1=0.0, scalar2=None, op0=ALU.is_ge)

            def conv2(n):
                s = n % 2
                a2, ot2 = A2[s], OT2[s]
                b2f = _flat(B2[s])
                final = n == IMGS_PER_CORE - 1
                for tp in range(4):
                    npair = 2 if tp < 3 else 1
                    nr = 8 * npair
                    r0 = 16 * tp
                    ps = psum2_pool.tile([P, 2, 512], F32)
                    for j in range(npair):
                        t = 2 * tp + j
                        for k in range(9):
                            ky, kx = divmod(k, 3)
                            off = (8 * t + ky) * WP + kx
                            nc.tensor.matmul(
                                ps[:, j, 0:TF], lhsT=WS2[:, P * k:P * (k + 1)],
                                rhs=b2f[:, off:off + TF],
                                start=(k == 0), stop=(k == 8))
                    t2 = stage_pool.tile([P, 2, TF], F32, tag="t2")
                    t23 = _flat(t2).rearrange("p (r c) -> p r c", c=WP)[:, 0:nr, 0:W]
                    nc.scalar.activation(
                        t2[:, 0:npair], ps[:, 0:npair, 0:TF],
                        ACTF.Identity, bias=b2, scale=s2)
                    nc.vector.tensor_tensor(
                        out=t23, in0=t23, in1=a2[:, r0:r0 + nr, :], op=ALU.add)
                    nc.gpsimd.tensor_scalar(
                        out=ot2[:, r0:r0 + nr, :], in0=t23,
                        scalar1=1.0, scalar2=-1.0, op0=ALU.min, op1=ALU.max)
                    if final and tp == 1:
                        # tail: flush the first half as soon as it's ready
                        h = 32 * W
                        nc.scalar.dma_start(
                            out=out_ch4(n, 0, half=0), in_=_flat(ot2)[0:64, 0:h])
                        nc.scalar.dma_start(
                            out=out_ch4(n, 2, half=0), in_=_flat(ot2)[64:128, 0:h])
                if final:
                    h = 32 * W
                    nc.scalar.dma_start(
                        out=out_ch4(n, 0, half=1), in_=_flat(ot2)[0:64, h:H * W])
                    nc.scalar.dma_start(
                        out=out_ch4(n, 2, half=1), in_=_flat(ot2)[64:128, h:H * W])
                else:
                    nc.scalar.dma_start(out=out_ch4(n, 0), in_=_flat(ot2)[0:64])
                    nc.scalar.dma_start(out=out_ch4(n, 2), in_=_flat(ot2)[64:128])

            # software pipeline across images: conv1(n+1) is emitted before
            # conv2(n) so the PE never stalls on the u2(n) dependency chain,
            # and image n+2's x_act load/binarize is emitted before conv2(n)
            # so its conv1 can start immediately after.
            def scoped(name, fn, *a):
                with nc.named_scope(name):
                    fn(*a)

            scoped("pa0", xa_load, 0)
            scoped("pa1", xa_load, 1)
            scoped("pi0", prelude_idle_loads, 0)
            scoped("pi1", prelude_idle_loads, 1)
            scoped("u1_0", u1, 0)
            scoped("c1_0", conv1, 0)      # embeds bias chunks 0 + u1(1)
            scoped("pa2", xa_load, 2)     # XA[0] free once c1_0 is emitted
            scoped("c1_1", conv1, 1)      # embeds bias chunks 1 + u1(2)
            scoped("pa3", xa_load, 3)
            for n in range(IMGS_PER_CORE - 2):
                scoped(f"c2_{n}", conv2, n)
                scoped(f"pi{n + 2}", prelude_idle_loads, n + 2)
                scoped(f"c1_{n + 2}", conv1, n + 2)  # embeds chunks + u1
            scoped(f"c2_{IMGS_PER_CORE - 2}", conv2, IMGS_PER_CORE - 2)
            scoped(f"c2_{IMGS_PER_CORE - 1}", conv2, IMGS_PER_CORE - 1)

    nc.compile()
    return nc


def _host_prep(w1, w2, bn1_gamma, bn1_beta, bn1_mean, bn1_var,
               bn2_gamma, bn2_beta, bn2_mean, bn2_var, move0_bias, move1_bias):
    f8 = np.float64
    bw1 = np.where(w1 >= 0, 1.0, -1.0).astype(f8)   # [co, ci, 3, 3]
    bw2 = np.where(w2 >= 0, 1.0, -1.0).astype(f8)

    # conv1 lhsT layout [ci, 9*co]: col k*128+co = bw1[co, ci, ky, kx]
    w1m = np.ascontiguousarray(
        bw1.transpose(1, 2, 3, 0).reshape(P, 9 * P)).astype(ml_dtypes.bfloat16)

    # conv2 channel permutation (both in and out sides)
    pidx = np.arange(P)
    chan = np.where(pidx < 64, 2 * pidx, 2 * (pidx - 64) + 1)  # partition -> x_act2 channel
    bw2p = bw2[np.ix_(chan, chan)]                  # [co', ci', 3, 3]
    w2m = np.ascontiguousarray(
        bw2p.transpose(1, 2, 3, 0).reshape(P, 9 * P)).astype(ml_dtypes.bfloat16)

    # u-domain: conv_sign = 2*conv_u - c0, c0 = sum of signed weights
    inv1 = bn1_gamma.astype(f8) / np.sqrt(bn1_var.astype(f8) + EPS)
    c0_1 = bw1.sum(axis=(1, 2, 3))
    s1 = 2.0 * inv1
    b1 = bn1_beta.astype(f8) - bn1_mean.astype(f8) * inv1 - inv1 * c0_1

    inv2 = (bn2_gamma.astype(f8) / np.sqrt(bn2_var.astype(f8) + EPS))[chan]
    c0_2 = bw2.sum(axis=(1, 2, 3))[chan]
    s2 = 2.0 * inv2
    b2 = bn2_beta.astype(f8)[chan] - bn2_mean.astype(f8)[chan] * inv2 - inv2 * c0_2

    cst = np.zeros((P, 16), np.float64)
    cst[:, 0] = s1
    cst[:, 1] = b1
    cst[:, 2] = s2
    cst[:, 3] = b2
    i = np.arange(64)
    cst[64:128, 4] = move1_bias[2 * i]
    cst[64:128, 7] = move0_bias[i]
    cst[64:128, 8] = move0_bias[64 + i] + move1_bias[2 * i + 1]
    return w1m, w2m, cst.astype(np.float32)


def kernel(x, w1, w2, bn1_gamma, bn1_beta, bn1_mean, bn1_var,
           bn2_gamma, bn2_beta, bn2_mean, bn2_var, move0_bias, move1_bias,
           _trace=False):
    x = np.asarray(x, np.float32)
    args = [np.asarray(a, np.float32) for a in (
        w1, w2, bn1_gamma, bn1_beta, bn1_mean, bn1_var,
        bn2_gamma, bn2_beta, bn2_mean, bn2_var, move0_bias, move1_bias)]
    w1m, w2m, cst = _host_prep(*args)

    if "nc" not in _CACHE:
        _CACHE["nc"] = _build()
    nc = _CACHE["nc"]

    in_maps = [
        {"xs": np.ascontiguousarray(x[IMGS_PER_CORE * c:IMGS_PER_CORE * (c + 1)]),
         "w1m": w1m, "w2m": w2m, "cst": cst}
        for c in range(NCORES)
    ]
    kw = {}
    if _trace:
        kw = dict(trace=True, trace_kwargs={"title": "basicblock"})
    res = bass_utils.run_bass_kernel_spmd(nc, in_maps, core_ids=list(range(NCORES)), **kw)
    out = np.concatenate([res.results[c]["out"] for c in range(NCORES)], axis=0)
    if _trace:
        _CACHE["last_results"] = res
    return out
